# revision 1
# baseline (speedup 1.0000x reference)
"""Trainium2 Bass kernel for causal multi-head attention.

Problem: B=2, T=4096, D=768, H=12 heads, d_k=64, causal mask.
Sharding: 8 cores = 2 batches x 4 head-groups (3 heads each).
Each core computes its batch's qkv projection (its heads only), flash-style
attention with transposed scores (S^T = k q^T, so softmax statistics land in
the matmul-friendly layout with no P-transposes), and a partial output
projection. Host sums the 4 head-group partials per batch and adds the
folded bias constant (v-bias @ W_out + b_out). The k-bias is dropped
entirely (softmax is invariant to per-query score shifts).

Self-contained: hardcodes all shapes; only imports the concourse runtime.
"""

import os
import sys

sys.path.insert(0, "/opt/trn_rl_repo")

from contextlib import ExitStack

import numpy as np

import concourse.bass as bass
import concourse.mybir as mybir
import concourse.tile as tile
from concourse import bacc
from concourse.bass_utils import run_bass_kernel_spmd

F32 = mybir.dt.float32

B, T, D = 2, 4096, 768
H, DK = 12, 64
HPC = 3          # heads per core
N_CORES = 8
ICH_W = 512      # i-chunk width (queries per outer step)
JB_W = 128       # j-block width (keys per matmul)

USE_DMA_TRANSPOSE = False   # fp32 xbar DMA-transpose unsupported (2-byte only)
USE_F32R = True             # run matmuls in float32r (1 cyc/row at N>=256 vs 4 for fp32)
F32R = mybir.dt.float32r
VPAD = 256                  # pad v-projection rhs to 256 cols so f32r hits fast path


MDT = F32R if USE_F32R else F32     # dtype for matmul operand tiles


def _r(ap):
    return ap


def build_program(t=T):
    """Build the SPMD Bass program for one core (all cores identical)."""
    n_ich = t // ICH_W          # i-chunks
    n_tch = t // 128            # t-chunks of 128 tokens
    KT = D // 128               # 6 contraction tiles for the projections

    nc = bacc.Bacc("TRN2", target_bir_lowering=False, debug=False,
                   num_devices=N_CORES)

    x_d = nc.dram_tensor("x", [t, D], F32, kind="ExternalInput").ap()
    # qk projection weights, 4 chunks of 128 output channels:
    # ch0=[q1|q2] ch1=[k1|k2] ch2=[q3|k3] ch3=[k3|q3]
    wqk_d = nc.dram_tensor("wqk", [D, 512], F32, kind="ExternalInput").ap()
    bqk_d = nc.dram_tensor("bqk", [512], F32, kind="ExternalInput").ap()
    wv_d = nc.dram_tensor("wv", [D, VPAD], F32, kind="ExternalInput").ap()
    wout_d = nc.dram_tensor("wout", [HPC * DK, D], F32, kind="ExternalInput").ap()
    out_d = nc.dram_tensor("out", [t, D], F32, kind="ExternalOutput").ap()

    with tile.TileContext(nc) as tc, ExitStack() as top:
        consts = top.enter_context(tc.tile_pool(name="consts", bufs=1))
        # persistent activations
        persist = top.enter_context(tc.tile_pool(name="persist", bufs=1))

        # q^T / k^T per chunk: [128, 4, t]
        qk_sb = persist.tile([128, 4, t], MDT)
        # v (natural layout) + ones column: [128, n_tch, HPC, 65]
        vaug_sb = persist.tile([128, n_tch, HPC, DK + 1], MDT)

        wqk_st = consts.tile([128, KT, 512], F32)
        nc.sync.dma_start(out=wqk_st, in_=wqk_d.rearrange("(kt p) c -> p kt c", p=128))
        wqk_sb = consts.tile([128, KT, 512], MDT)
        nc.vector.tensor_copy(wqk_sb, wqk_st)
        bqk_sb = consts.tile([128, 4], F32)
        nc.sync.dma_start(out=bqk_sb, in_=bqk_d.rearrange("(ch p) -> p ch", p=128))
        wv_st = consts.tile([128, KT, VPAD], F32)
        nc.sync.dma_start(out=wv_st, in_=wv_d.rearrange("(kt p) c -> p kt c", p=128))
        wv_sb = consts.tile([128, KT, VPAD], MDT)
        nc.vector.tensor_copy(wv_sb, wv_st)
        wout_st = consts.tile([64, HPC, D], F32)
        nc.sync.dma_start(out=wout_st, in_=wout_d.rearrange("(h p) m -> p h m", p=64))
        wout_sb = consts.tile([64, HPC, D], MDT)
        nc.vector.tensor_copy(wout_sb, wout_st)

        ones3 = consts.tile([128, 3], F32)
        nc.vector.memset(ones3, 1.0)

        identity = None
        if not USE_DMA_TRANSPOSE:
            from concourse.masks import make_identity
            identity = consts.tile([128, 128], F32)
            make_identity(nc, identity)

        # ---------------- Phase 1+2: x^T (streamed) + projections ----------
        with tc.tile_pool(name="xt", bufs=3) as xtp, \
             tc.tile_pool(name="xn", bufs=3) as xnp, \
             tc.tile_pool(name="p2ps", bufs=2, space="PSUM") as p2ps, \
             tc.tile_pool(name="p2ps_v", bufs=2, space="PSUM") as p2psv:
            for ich in range(n_ich):
                i0 = ich * ICH_W
                xt = xtp.tile([128, KT, ICH_W], MDT, tag="xt")
                if USE_DMA_TRANSPOSE:
                    for c in range(D // 64):
                        nc.sync.dma_start(
                            out=xt[(c % 2) * 64:(c % 2) * 64 + 64, c // 2, :],
                            in_=x_d[i0:i0 + ICH_W, c * 64:(c + 1) * 64],
                            transpose=True,
                        )
                else:
                    for tl in range(ICH_W // 128):
                        xn = xnp.tile([128, D], F32, tag="xn")
                        nc.sync.dma_start(
                            out=xn, in_=x_d[i0 + tl * 128:i0 + (tl + 1) * 128, :])
                        for kt in range(KT):
                            tps = p2ps.tile([128, 128], F32, tag="tr",
                                            space="PSUM")
                            nc.tensor.transpose(
                                tps, xn[:, kt * 128:(kt + 1) * 128], identity)
                            nc.vector.tensor_copy(
                                xt[:, kt, tl * 128:(tl + 1) * 128], tps)
                # q^T/k^T chunks for this i-range
                for ch in range(4):
                    qps = p2ps.tile([128, ICH_W], F32, tag="qk", space="PSUM")
                    for kt in range(KT):
                        nc.tensor.matmul(
                            qps,
                            lhsT=_r(wqk_sb[:, kt, ch * 128:(ch + 1) * 128]),
                            rhs=_r(xt[:, kt, :]),
                            start=(kt == 0), stop=(kt == KT - 1),
                        )
                    nc.vector.tensor_scalar_add(
                        qk_sb[:, ch, i0:i0 + ICH_W], qps, bqk_sb[:, ch:ch + 1])
                # v natural for the 4 t-chunks in this i-range
                for tl in range(ICH_W // 128):
                    tch = ich * (ICH_W // 128) + tl
                    vps = p2psv.tile([128, VPAD], F32, tag="v", space="PSUM")
                    for kt in range(KT):
                        nc.tensor.matmul(
                            vps,
                            lhsT=_r(xt[:, kt, tl * 128:(tl + 1) * 128]),
                            rhs=_r(wv_sb[:, kt, :]),
                            start=(kt == 0), stop=(kt == KT - 1),
                        )
                    nc.vector.tensor_copy(
                        vaug_sb[:, tch, :, 0:DK],
                        vps[:, 0:HPC * DK].rearrange("p (h d) -> p h d", h=HPC),
                    )
                    nc.vector.tensor_copy(
                        vaug_sb[:, tch, :, DK:DK + 1],
                        ones3.rearrange("p (a b) -> p a b", b=1))

        # head views: (qT, kT) partition slices + base partition for pairing
        # h0: q=ch0[0:64]   k=ch1[0:64]    (base 0)
        # h1: q=ch0[64:128] k=ch1[64:128]  (base 64)
        # h2 even jb: q=ch2[0:64]  k=ch3[0:64]   (base 0)
        # h2 odd  jb: q=ch3[64:128] k=ch2[64:128] (base 64)

        # ---------------- Phase 3: attention + out projection ---------------
        with tc.tile_pool(name="stps", bufs=2, space="PSUM") as stps, \
             tc.tile_pool(name="cps", bufs=2, space="PSUM") as cpsp, \
             tc.tile_pool(name="ops", bufs=1, space="PSUM") as opsp, \
             tc.tile_pool(name="pt", bufs=3) as ptp, \
             tc.tile_pool(name="ctxn", bufs=3) as ctxp, \
             tc.tile_pool(name="small", bufs=4) as smp, \
             tc.tile_pool(name="outsb", bufs=2) as outp:
            for ich in range(n_ich):
                i0 = ich * ICH_W
                njb = (i0 + ICH_W) // JB_W     # causal: j-blocks 0..njb-1
                ctxn = {}

                # ---- pass A: heads 0 and 1, row-group paired ----
                cps0 = cpsp.tile([65, ICH_W], F32, tag="cps", space="PSUM")
                cps1 = cpsp.tile([65, ICH_W], F32, tag="cps", space="PSUM")
                for jb in range(njb):           # 1 j-block x 2 heads per group
                    j0 = jb * JB_W
                    st = stps.tile([128, 2, ICH_W], F32, tag="st", space="PSUM")
                    # h0 at rows 0-63, h1 at rows 64-127: concurrent MMs
                    nc.tensor.matmul(
                        st[:, 0, :],
                        lhsT=_r(qk_sb[0:64, 1, j0:j0 + JB_W]),
                        rhs=_r(qk_sb[0:64, 0, i0:i0 + ICH_W]),
                        start=True, stop=True)
                    nc.tensor.matmul(
                        st[:, 1, :],
                        lhsT=_r(qk_sb[64:128, 1, j0:j0 + JB_W]),
                        rhs=_r(qk_sb[64:128, 0, i0:i0 + ICH_W]),
                        start=True, stop=True)
                    pt = ptp.tile([128, 2, ICH_W], MDT, tag="pt")
                    nc.scalar.activation(pt, st,
                                         mybir.ActivationFunctionType.Exp,
                                         bias=0.0, scale=1.0 / np.sqrt(DK))
                    s = jb - (njb - 4)          # diag position if >= 0
                    if s >= 0:
                        w = 128 * (s + 1)
                        for hh in range(2):
                            nc.gpsimd.affine_select(
                                out=pt[:, hh, 0:w],
                                in_=pt[:, hh, 0:w],
                                compare_op=mybir.AluOpType.is_ge,
                                fill=0.0, base=-128 * s,
                                pattern=[[1, w]], channel_multiplier=-1)
                    nc.tensor.matmul(
                        cps0, lhsT=_r(vaug_sb[:, jb, 0, :]),
                        rhs=_r(pt[:, 0, :]),
                        start=(jb == 0), stop=(jb == njb - 1))
                    nc.tensor.matmul(
                        cps1, lhsT=_r(vaug_sb[:, jb, 1, :]),
                        rhs=_r(pt[:, 1, :]),
                        start=(jb == 0), stop=(jb == njb - 1))

                # ---- normalize h0/h1 now so their cps slots free before
                # pass B allocates cps2 (cps pool has bufs=2) ----
                for h, cps in ((0, cps0), (1, cps1)):
                    recip = smp.tile([1, ICH_W], F32, tag="recip")
                    nc.vector.reciprocal(recip, cps[64:65, :])
                    rb = smp.tile([64, ICH_W], F32, tag="rb")
                    nc.gpsimd.partition_broadcast(rb, recip)
                    cn = ctxp.tile([64, ICH_W], MDT, tag="ctxn")
                    nc.vector.tensor_mul(cn, cps[0:64, :], rb)
                    ctxn[h] = cn

                # ---- pass B: head 2, alternating row groups ----
                cps2 = cpsp.tile([65, ICH_W], F32, tag="cps", space="PSUM")
                for grp in range(njb // 2):     # 2 j-blocks per psum group
                    st = stps.tile([128, 2, ICH_W], F32, tag="st", space="PSUM")
                    for jj in range(2):
                        jb = grp * 2 + jj
                        j0 = jb * JB_W
                        if jb % 2 == 0:
                            lhsT = qk_sb[0:64, 3, j0:j0 + JB_W]
                            rhs = qk_sb[0:64, 2, i0:i0 + ICH_W]
                        else:
                            lhsT = qk_sb[64:128, 2, j0:j0 + JB_W]
                            rhs = qk_sb[64:128, 3, i0:i0 + ICH_W]
                        nc.tensor.matmul(st[:, jj, :], lhsT=_r(lhsT),
                                         rhs=_r(rhs), start=True, stop=True)
                    pt = ptp.tile([128, 2, ICH_W], MDT, tag="pt")
                    nc.scalar.activation(pt, st,
                                         mybir.ActivationFunctionType.Exp,
                                         bias=0.0, scale=1.0 / np.sqrt(DK))
                    for jj in range(2):
                        jb = grp * 2 + jj
                        s = jb - (njb - 4)
                        if s >= 0:
                            w = 128 * (s + 1)
                            nc.gpsimd.affine_select(
                                out=pt[:, jj, 0:w], in_=pt[:, jj, 0:w],
                                compare_op=mybir.AluOpType.is_ge,
                                fill=0.0, base=-128 * s,
                                pattern=[[1, w]], channel_multiplier=-1)
                    for jj in range(2):
                        jb = grp * 2 + jj
                        nc.tensor.matmul(
                            cps2, lhsT=_r(vaug_sb[:, jb, 2, :]),
                            rhs=_r(pt[:, jj, :]),
                            start=(jb == 0), stop=(jb == njb - 1))

                # ---- normalize head 2 ----
                for h, cps in ((2, cps2),):
                    recip = smp.tile([1, ICH_W], F32, tag="recip")
                    nc.vector.reciprocal(recip, cps[64:65, :])
                    rb = smp.tile([64, ICH_W], F32, tag="rb")
                    nc.gpsimd.partition_broadcast(rb, recip)
                    cn = ctxp.tile([64, ICH_W], MDT, tag="ctxn")
                    nc.vector.tensor_mul(cn, cps[0:64, :], rb)
                    ctxn[h] = cn

                # ---- partial out projection for this i-chunk ----
                for tsub in range(ICH_W // 128):
                    ops = opsp.tile([128, D], F32, tag="ops", space="PSUM")
                    for h in range(HPC):
                        for mi, (m0, m1) in enumerate(((0, 512), (512, D))):
                            nc.tensor.matmul(
                                ops[:, m0:m1],
                                lhsT=_r(ctxn[h][:, tsub * 128:(tsub + 1) * 128]),
                                rhs=_r(wout_sb[:, h, m0:m1]),
                                start=(h == 0), stop=(h == HPC - 1))
                    osb = outp.tile([128, D], F32, tag="osb")
                    nc.vector.tensor_copy(osb, ops)
                    nc.sync.dma_start(
                        out=out_d[i0 + tsub * 128:i0 + (tsub + 1) * 128, :],
                        in_=osb)

    nc.compile()
    return nc


def make_core_inputs(x_b, W_qkv, b_qkv, W_out, hg):
    """Host-side weight slicing/permutation for one head-group hg (0..3)."""
    heads = [hg * HPC + i for i in range(HPC)]
    # W_qkv last-dim layout: c = h*192 + s*64 + d  (s: 0=q 1=k 2=v)
    def cols(h, s):
        return slice(h * 192 + s * 64, h * 192 + s * 64 + 64)

    q = [np.asarray(W_qkv[:, cols(h, 0)]) for h in heads]
    k = [np.asarray(W_qkv[:, cols(h, 1)]) for h in heads]
    v = [np.asarray(W_qkv[:, cols(h, 2)]) for h in heads]
    bq = [np.asarray(b_qkv[cols(h, 0)]) for h in heads]

    wqk = np.concatenate([q[0], q[1], k[0], k[1], q[2], k[2], k[2], q[2]],
                         axis=1).astype(np.float32)
    z = np.zeros(64, np.float32)
    bqk = np.concatenate([bq[0], bq[1], z, z, bq[2], z, z, bq[2]]).astype(
        np.float32)
    wv = np.concatenate(v, axis=1).astype(np.float32)
    wv = np.pad(wv, ((0, 0), (0, 256 - wv.shape[1])))
    wout = np.concatenate(
        [np.asarray(W_out[h * DK:(h + 1) * DK, :]) for h in heads],
        axis=0).astype(np.float32)
    return {
        "x": np.ascontiguousarray(np.asarray(x_b, np.float32)),
        "wqk": np.ascontiguousarray(wqk),
        "bqk": np.ascontiguousarray(bqk),
        "wv": np.ascontiguousarray(wv),
        "wout": np.ascontiguousarray(wout),
    }


_CACHE = {}


def _get_program(t=T):
    if t not in _CACHE:
        _CACHE[t] = build_program(t)
    return _CACHE[t]


def run_cores(inputs, t=T, trace=False):
    nc = _get_program(t)
    x = np.asarray(inputs["x"], np.float32)
    in_maps = []
    for core in range(N_CORES):
        b, hg = core // 4, core % 4
        in_maps.append(make_core_inputs(x[b], inputs["W_qkv"],
                                        inputs["b_qkv"], inputs["W_out"], hg))
    res = run_bass_kernel_spmd(nc, in_maps, list(range(N_CORES)), trace=trace)
    return res


def gather(inputs, results):
    b_qkv = np.asarray(inputs["b_qkv"], np.float32)
    W_out = np.asarray(inputs["W_out"], np.float32)
    b_out = np.asarray(inputs["b_out"], np.float32)
    bv = np.concatenate([b_qkv[h * 192 + 128:h * 192 + 192] for h in range(H)])
    fold = bv @ W_out + b_out                      # [D]
    t = results[0]["out"].shape[0]
    out = np.zeros((B, t, D), np.float32)
    for core in range(N_CORES):
        out[core // 4] += results[core]["out"]
    out += fold[None, None, :]
    return out


def kernel(**inputs):
    res = run_cores(inputs)
    return gather(inputs, res.results)


if __name__ == "__main__":
    # smoke test with random data
    rng = np.random.default_rng(0)
    inputs = {
        "x": rng.standard_normal((B, T, D), dtype=np.float32),
        "mask": np.triu(np.ones((T, T), dtype=bool), k=1),
        "W_qkv": (rng.standard_normal((D, 3 * D), dtype=np.float32)
                  / np.sqrt(D)),
        "b_qkv": rng.standard_normal(3 * D, dtype=np.float32) * 0.02,
        "W_out": (rng.standard_normal((D, D), dtype=np.float32)
                  / np.sqrt(D)),
        "b_out": rng.standard_normal(D, dtype=np.float32) * 0.02,
    }
    out = kernel(**inputs)
    print(out.shape, out.dtype)



# revision 2
# speedup vs baseline: 1.3753x; 1.3753x over previous
"""Trainium2 Bass kernel for causal multi-head attention.

Problem: B=2, T=4096, D=768, H=12 heads, d_k=64, causal mask.
Sharding: 8 cores = 2 batches x 4 head-groups (3 heads each).

v2 design (all-bf16 on device):
- Host ships x^T (pre-transposed, bf16) so the kernel needs no PE
  transposes; weights are pre-sliced/concatenated per head-group and cast
  to bf16 on host.
- One fused loop per 512-query i-chunk: project qk^T/v for the chunk's
  tokens, then flash-style causal attention with transposed scores
  (S^T = k q^T so softmax stats land matmul-friendly), then a partial
  out-projection.  Projections of chunk i overlap attention of chunk i-1
  through the Tile scheduler.
- Causal handling at 128-block granularity: fully-masked columns are
  skipped in the score matmul / exp / pv matmul; the single true-diagonal
  128x128 block is masked by a precomputed triangular bf16 tile via DVE
  tensor_mul (no gpsimd affine_select on the hot path).
- Host sums the 4 head-group partials per batch (bf16 partials) and adds
  the folded bias constant (v-bias @ W_out + b_out).  The k-bias is
  dropped (softmax is invariant to per-query score shifts).

Self-contained: hardcodes all shapes; only imports the concourse runtime.
"""

import sys

sys.path.insert(0, "/opt/trn_rl_repo")

from contextlib import ExitStack

import numpy as np
import ml_dtypes

import concourse.bass as bass
import concourse.mybir as mybir
import concourse.tile as tile
from concourse import bacc
from concourse.bass_utils import run_bass_kernel_spmd

F32 = mybir.dt.float32
BF16 = mybir.dt.bfloat16
NPBF16 = ml_dtypes.bfloat16

B, T, D = 2, 4096, 768
H, DK = 12, 64
HPC = 3          # heads per core
N_CORES = 8
ICH_W = 512      # i-chunk width (queries per outer step)
JB_W = 128       # j-block width (keys per matmul)
KT = D // 128    # 6 contraction tiles for the projections


def build_program(t=T):
    """Build the SPMD Bass program for one core (all cores identical)."""
    n_ich = t // ICH_W
    n_tch = t // 128

    nc = bacc.Bacc("TRN2", target_bir_lowering=False, debug=False,
                   num_devices=N_CORES)

    # x^T: [D, t] bf16, row-major (row stride t)
    xt_d = nc.dram_tensor("xt", [D, t], BF16, kind="ExternalInput").ap()
    # qk projection weights, 4 chunks of 128 output channels:
    # ch0=[q1|q2] ch1=[k1|k2] ch2=[q3|k3] ch3=[k3|q3]
    wqk_d = nc.dram_tensor("wqk", [D, 512], BF16, kind="ExternalInput").ap()
    bqk_d = nc.dram_tensor("bqk", [512], F32, kind="ExternalInput").ap()
    wv_d = nc.dram_tensor("wv", [D, HPC * DK], BF16, kind="ExternalInput").ap()
    wout_d = nc.dram_tensor("wout", [HPC * DK, D], BF16,
                            kind="ExternalInput").ap()
    out_d = nc.dram_tensor("out", [t, D], BF16, kind="ExternalOutput").ap()

    with tile.TileContext(nc) as tc, ExitStack() as top:
        consts = top.enter_context(tc.tile_pool(name="consts", bufs=1))
        persist = top.enter_context(tc.tile_pool(name="persist", bufs=1))

        # q^T / k^T per chunk: [128, 4, t] bf16
        qk_sb = persist.tile([128, 4, t], BF16)
        # v (natural layout) + ones column: [128, n_tch, HPC, 65] bf16
        vaug_sb = persist.tile([128, n_tch, HPC, DK + 1], BF16)

        wqk_sb = consts.tile([128, KT, 512], BF16)
        nc.sync.dma_start(out=wqk_sb,
                          in_=wqk_d.rearrange("(kt p) c -> p kt c", p=128))
        bqk_sb = consts.tile([128, 4], F32)
        nc.sync.dma_start(out=bqk_sb, in_=bqk_d.rearrange("(ch p) -> p ch",
                                                          p=128))
        wv_sb = consts.tile([128, KT, HPC * DK], BF16)
        nc.sync.dma_start(out=wv_sb,
                          in_=wv_d.rearrange("(kt p) c -> p kt c", p=128))
        wout_sb = consts.tile([64, HPC, D], BF16)
        nc.sync.dma_start(out=wout_sb,
                          in_=wout_d.rearrange("(h p) m -> p h m", p=64))

        # ones column for the v-augmentation (denominator row)
        ones3 = consts.tile([128, HPC], BF16)
        nc.vector.memset(ones3, 1.0)
        # lower-triangular-inclusive multiplicative mask for the diagonal
        # 128x128 block: tri[p, c] = 1 if p <= c else 0
        tri = consts.tile([128, 128], BF16)
        nc.vector.memset(tri, 1.0)
        nc.gpsimd.affine_select(
            out=tri, in_=tri, compare_op=mybir.AluOpType.is_ge,
            fill=0.0, base=0, pattern=[[1, 128]], channel_multiplier=-1)

        with tc.tile_pool(name="xtp", bufs=3) as xtp, \
             tc.tile_pool(name="work_ps", bufs=2, space="PSUM") as workp, \
             tc.tile_pool(name="stps", bufs=2, space="PSUM") as stps, \
             tc.tile_pool(name="cps", bufs=2, space="PSUM") as cpsp, \
             tc.tile_pool(name="pt", bufs=3) as ptp, \
             tc.tile_pool(name="ctxn", bufs=3) as ctxp, \
             tc.tile_pool(name="small", bufs=4) as smp, \
             tc.tile_pool(name="outsb", bufs=3) as outp:
            for ich in range(n_ich):
                i0 = ich * ICH_W

                # ---- projections for this i-chunk's tokens ----
                xt = xtp.tile([128, KT, ICH_W], BF16, tag="xt")
                nc.sync.dma_start(
                    out=xt,
                    in_=xt_d[:, i0:i0 + ICH_W].rearrange(
                        "(kt p) i -> p kt i", p=128))
                for ch in range(4):
                    qps = workp.tile([128, 512], F32, tag="w", space="PSUM")
                    for kt in range(KT):
                        nc.tensor.matmul(
                            qps,
                            lhsT=wqk_sb[:, kt, ch * 128:(ch + 1) * 128],
                            rhs=xt[:, kt, :],
                            start=(kt == 0), stop=(kt == KT - 1),
                        )
                    nc.vector.tensor_scalar_add(
                        qk_sb[:, ch, i0:i0 + ICH_W], qps,
                        bqk_sb[:, ch:ch + 1])
                for tl in range(ICH_W // 128):
                    tch = ich * (ICH_W // 128) + tl
                    vps = workp.tile([128, 512], F32, tag="w", space="PSUM")
                    for kt in range(KT):
                        nc.tensor.matmul(
                            vps[:, 0:HPC * DK],
                            lhsT=xt[:, kt, tl * 128:(tl + 1) * 128],
                            rhs=wv_sb[:, kt, :],
                            start=(kt == 0), stop=(kt == KT - 1),
                        )
                    nc.vector.tensor_copy(
                        vaug_sb[:, tch, :, 0:DK],
                        vps[:, 0:HPC * DK].rearrange("p (h d) -> p h d",
                                                     h=HPC),
                    )
                    nc.vector.tensor_copy(
                        vaug_sb[:, tch, :, DK:DK + 1],
                        ones3.rearrange("p (a b) -> p a b", b=1))

                njb = (i0 + ICH_W) // JB_W     # causal: j-blocks 0..njb-1
                ctxn = {}

                # head views: (qT, kT) partition slices
                # h0: q=ch0[0:64]   k=ch1[0:64]
                # h1: q=ch0[64:128] k=ch1[64:128]
                # h2 even jb: q=ch2[0:64]  k=ch3[0:64]
                # h2 odd  jb: q=ch3[64:128] k=ch2[64:128]

                # ---- pass A: heads 0 and 1, row-group paired ----
                cps0 = cpsp.tile([65, ICH_W], F32, tag="cps", space="PSUM")
                cps1 = cpsp.tile([65, ICH_W], F32, tag="cps", space="PSUM")
                for jb in range(njb):
                    j0 = jb * JB_W
                    s = jb - (njb - 4)          # diag position if >= 0
                    w0 = 128 * s if s > 0 else 0   # fully-masked columns
                    st = stps.tile([128, 2, ICH_W], F32, tag="st",
                                   space="PSUM")
                    nc.tensor.matmul(
                        st[:, 0, w0:],
                        lhsT=qk_sb[0:64, 1, j0:j0 + JB_W],
                        rhs=qk_sb[0:64, 0, i0 + w0:i0 + ICH_W],
                        start=True, stop=True)
                    nc.tensor.matmul(
                        st[:, 1, w0:],
                        lhsT=qk_sb[64:128, 1, j0:j0 + JB_W],
                        rhs=qk_sb[64:128, 0, i0 + w0:i0 + ICH_W],
                        start=True, stop=True)
                    pt = ptp.tile([128, 2, ICH_W], BF16, tag="pt")
                    nc.scalar.activation(pt[:, :, w0:], st[:, :, w0:],
                                         mybir.ActivationFunctionType.Exp,
                                         bias=0.0, scale=1.0 / np.sqrt(DK))
                    if s >= 0:
                        # true-diagonal block: multiplicative triangular mask
                        for hh in range(2):
                            nc.vector.tensor_mul(
                                pt[:, hh, w0:w0 + 128],
                                pt[:, hh, w0:w0 + 128], tri)
                    nc.tensor.matmul(
                        cps0[:, w0:], lhsT=vaug_sb[:, jb, 0, :],
                        rhs=pt[:, 0, w0:],
                        start=(jb == 0), stop=(jb == njb - 1))
                    nc.tensor.matmul(
                        cps1[:, w0:], lhsT=vaug_sb[:, jb, 1, :],
                        rhs=pt[:, 1, w0:],
                        start=(jb == 0), stop=(jb == njb - 1))

                # ---- normalize h0/h1 (frees cps slots before pass B) ----
                for h, cps in ((0, cps0), (1, cps1)):
                    recip = smp.tile([1, ICH_W], F32, tag="recip")
                    nc.vector.reciprocal(recip, cps[64:65, :])
                    rb = smp.tile([64, ICH_W], F32, tag="rb")
                    nc.gpsimd.partition_broadcast(rb, recip)
                    cn = ctxp.tile([64, ICH_W], BF16, tag="ctxn")
                    nc.vector.tensor_mul(cn, cps[0:64, :], rb)
                    ctxn[h] = cn

                # ---- pass B: head 2, alternating row groups ----
                cps2 = cpsp.tile([65, ICH_W], F32, tag="cps", space="PSUM")
                for grp in range(njb // 2):
                    st = stps.tile([128, 2, ICH_W], F32, tag="st",
                                   space="PSUM")
                    pt = ptp.tile([128, 2, ICH_W], BF16, tag="pt")
                    for jj in range(2):
                        jb = grp * 2 + jj
                        j0 = jb * JB_W
                        s = jb - (njb - 4)
                        w0 = 128 * s if s > 0 else 0
                        if jb % 2 == 0:
                            lhsT = qk_sb[0:64, 3, j0:j0 + JB_W]
                            rhs = qk_sb[0:64, 2, i0 + w0:i0 + ICH_W]
                        else:
                            lhsT = qk_sb[64:128, 2, j0:j0 + JB_W]
                            rhs = qk_sb[64:128, 3, i0 + w0:i0 + ICH_W]
                        nc.tensor.matmul(st[:, jj, w0:], lhsT=lhsT, rhs=rhs,
                                         start=True, stop=True)
                        nc.scalar.activation(
                            pt[:, jj, w0:], st[:, jj, w0:],
                            mybir.ActivationFunctionType.Exp,
                            bias=0.0, scale=1.0 / np.sqrt(DK))
                        if s >= 0:
                            nc.vector.tensor_mul(
                                pt[:, jj, w0:w0 + 128],
                                pt[:, jj, w0:w0 + 128], tri)
                    for jj in range(2):
                        jb = grp * 2 + jj
                        s = jb - (njb - 4)
                        w0 = 128 * s if s > 0 else 0
                        nc.tensor.matmul(
                            cps2[:, w0:], lhsT=vaug_sb[:, jb, 2, :],
                            rhs=pt[:, jj, w0:],
                            start=(jb == 0), stop=(jb == njb - 1))

                # ---- normalize head 2 ----
                for h, cps in ((2, cps2),):
                    recip = smp.tile([1, ICH_W], F32, tag="recip")
                    nc.vector.reciprocal(recip, cps[64:65, :])
                    rb = smp.tile([64, ICH_W], F32, tag="rb")
                    nc.gpsimd.partition_broadcast(rb, recip)
                    cn = ctxp.tile([64, ICH_W], BF16, tag="ctxn")
                    nc.vector.tensor_mul(cn, cps[0:64, :], rb)
                    ctxn[h] = cn

                # ---- partial out projection for this i-chunk ----
                for tsub in range(ICH_W // 128):
                    osb = outp.tile([128, D], BF16, tag="osb")
                    for mi, (m0, m1) in enumerate(((0, 384), (384, D))):
                        ops = workp.tile([128, 512], F32, tag="w",
                                         space="PSUM")
                        for h in range(HPC):
                            nc.tensor.matmul(
                                ops[:, 0:m1 - m0],
                                lhsT=ctxn[h][:, tsub * 128:(tsub + 1) * 128],
                                rhs=wout_sb[:, h, m0:m1],
                                start=(h == 0), stop=(h == HPC - 1))
                        nc.vector.tensor_copy(osb[:, m0:m1],
                                              ops[:, 0:m1 - m0])
                    nc.sync.dma_start(
                        out=out_d[i0 + tsub * 128:i0 + (tsub + 1) * 128, :],
                        in_=osb)

    nc.compile()
    return nc


def _to_bf16(a):
    return np.ascontiguousarray(np.asarray(a).astype(NPBF16))


def make_core_inputs(xt_b16, W_qkv, b_qkv, W_out, hg):
    """Host-side weight slicing/permutation for one head-group hg (0..3).

    ``xt_b16``: pre-transposed+cast [D, t] bf16 (shared across the 4 cores
    of a batch — pass the same array; no per-core copy).
    """
    heads = [hg * HPC + i for i in range(HPC)]
    # W_qkv last-dim layout: c = h*192 + s*64 + d  (s: 0=q 1=k 2=v)
    def cols(h, s):
        return slice(h * 192 + s * 64, h * 192 + s * 64 + 64)

    q = [np.asarray(W_qkv[:, cols(h, 0)]) for h in heads]
    k = [np.asarray(W_qkv[:, cols(h, 1)]) for h in heads]
    v = [np.asarray(W_qkv[:, cols(h, 2)]) for h in heads]
    bq = [np.asarray(b_qkv[cols(h, 0)], np.float32) for h in heads]

    wqk = np.concatenate([q[0], q[1], k[0], k[1], q[2], k[2], k[2], q[2]],
                         axis=1)
    z = np.zeros(64, np.float32)
    bqk = np.concatenate([bq[0], bq[1], z, z, bq[2], z, z, bq[2]]).astype(
        np.float32)
    wv = np.concatenate(v, axis=1)
    wout = np.concatenate(
        [np.asarray(W_out[h * DK:(h + 1) * DK, :]) for h in heads], axis=0)
    return {
        "xt": xt_b16,
        "wqk": _to_bf16(wqk),
        "bqk": np.ascontiguousarray(bqk),
        "wv": _to_bf16(wv),
        "wout": _to_bf16(wout),
    }


_CACHE = {}


def _get_program(t=T):
    if t not in _CACHE:
        _CACHE[t] = build_program(t)
    return _CACHE[t]


def run_cores(inputs, t=T, trace=False):
    nc = _get_program(t)
    x = np.asarray(inputs["x"], np.float32)
    xt_b16 = [np.ascontiguousarray(x[b].T.astype(NPBF16)) for b in range(B)]
    in_maps = []
    for core in range(N_CORES):
        b, hg = core // 4, core % 4
        in_maps.append(make_core_inputs(xt_b16[b], inputs["W_qkv"],
                                        inputs["b_qkv"], inputs["W_out"], hg))
    res = run_bass_kernel_spmd(nc, in_maps, list(range(N_CORES)), trace=trace)
    return res


def gather(inputs, results):
    b_qkv = np.asarray(inputs["b_qkv"], np.float32)
    W_out = np.asarray(inputs["W_out"], np.float32)
    b_out = np.asarray(inputs["b_out"], np.float32)
    bv = np.concatenate([b_qkv[h * 192 + 128:h * 192 + 192] for h in range(H)])
    fold = bv @ W_out + b_out                      # [D]
    t = results[0]["out"].shape[0]
    out = np.zeros((B, t, D), np.float32)
    for core in range(N_CORES):
        out[core // 4] += np.asarray(results[core]["out"], np.float32)
    out += fold[None, None, :]
    return out


def kernel(**inputs):
    res = run_cores(inputs)
    return gather(inputs, res.results)


if __name__ == "__main__":
    # smoke test with random data
    rng = np.random.default_rng(0)
    inputs = {
        "x": rng.standard_normal((B, T, D), dtype=np.float32),
        "mask": np.triu(np.ones((T, T), dtype=bool), k=1),
        "W_qkv": (rng.standard_normal((D, 3 * D), dtype=np.float32)
                  / np.sqrt(D)),
        "b_qkv": rng.standard_normal(3 * D).astype(np.float32) * 0.02,
        "W_out": (rng.standard_normal((D, D), dtype=np.float32)
                  / np.sqrt(D)),
        "b_out": rng.standard_normal(D).astype(np.float32) * 0.02,
    }
    out = kernel(**inputs)
    print(out.shape, out.dtype)


# revision 5
# speedup vs baseline: 1.7011x; 1.2369x over previous
"""Trainium2 Bass kernel for causal multi-head attention.

Problem: B=2, T=4096, D=768, H=12 heads, d_k=64, causal mask.
Sharding: 8 cores = 2 batches x 4 head-groups (3 heads each).

v2 design (all-bf16 on device):
- Host ships x^T (pre-transposed, bf16) so the kernel needs no PE
  transposes; weights are pre-sliced/concatenated per head-group and cast
  to bf16 on host.
- One fused loop per 512-query i-chunk: project qk^T/v for the chunk's
  tokens, then flash-style causal attention with transposed scores
  (S^T = k q^T so softmax stats land matmul-friendly), then a partial
  out-projection.  Projections of chunk i overlap attention of chunk i-1
  through the Tile scheduler.
- Causal handling at 128-block granularity: fully-masked columns are
  skipped in the score matmul / exp / pv matmul; the single true-diagonal
  128x128 block is masked by a precomputed triangular bf16 tile via DVE
  tensor_mul (no gpsimd affine_select on the hot path).
- Host sums the 4 head-group partials per batch (bf16 partials) and adds
  the folded bias constant (v-bias @ W_out + b_out).  The k-bias is
  dropped (softmax is invariant to per-query score shifts).

Self-contained: hardcodes all shapes; only imports the concourse runtime.
"""

import sys

sys.path.insert(0, "/opt/trn_rl_repo")

from contextlib import ExitStack

import numpy as np
import ml_dtypes

import concourse.bass as bass
import concourse.mybir as mybir
import concourse.tile as tile
from concourse import bacc
from concourse.bass_utils import run_bass_kernel_spmd

F32 = mybir.dt.float32
BF16 = mybir.dt.bfloat16
NPBF16 = ml_dtypes.bfloat16

B, T, D = 2, 4096, 768
H, DK = 12, 64
HPC = 3          # heads per core
N_CORES = 8
ICH_W = 512      # i-chunk width (queries per outer step)
JB_W = 128       # j-block width (keys per matmul)
KT = D // 128    # 6 contraction tiles for the projections


def build_program(t=T):
    """Build the SPMD Bass program for one core (all cores identical)."""
    n_ich = t // ICH_W
    n_tch = t // 128

    nc = bacc.Bacc("TRN2", target_bir_lowering=False, debug=False,
                   num_devices=N_CORES)

    # x^T: [D, t] bf16, row-major (row stride t)
    xt_d = nc.dram_tensor("xt", [D, t], BF16, kind="ExternalInput").ap()
    # qk projection weights, 4 chunks of 128 output channels:
    # ch0=[q1|q2] ch1=[k1|k2] ch2=[q3|k3] ch3=[k3|q3]
    wqk_d = nc.dram_tensor("wqk", [D, 512], BF16, kind="ExternalInput").ap()
    bqk_d = nc.dram_tensor("bqk", [512], F32, kind="ExternalInput").ap()
    wv_d = nc.dram_tensor("wv", [D, HPC * DK], BF16, kind="ExternalInput").ap()
    wout_d = nc.dram_tensor("wout", [HPC * DK, D], BF16,
                            kind="ExternalInput").ap()
    out_d = nc.dram_tensor("out", [t, D], BF16, kind="ExternalOutput").ap()

    with tile.TileContext(nc) as tc, ExitStack() as top:
        consts = top.enter_context(tc.tile_pool(name="consts", bufs=1))
        persist = top.enter_context(tc.tile_pool(name="persist", bufs=1))

        # q^T / k^T per chunk: [128, 4, t] bf16
        qk_sb = persist.tile([128, 4, t], BF16)
        # v (natural layout) + ones column: [128, n_tch, HPC, 65] bf16
        vaug_sb = persist.tile([128, n_tch, HPC, DK + 1], BF16)

        wqk_sb = consts.tile([128, KT, 512], BF16)
        nc.sync.dma_start(out=wqk_sb,
                          in_=wqk_d.rearrange("(kt p) c -> p kt c", p=128))
        bqk_sb = consts.tile([128, 4], F32)
        nc.sync.dma_start(out=bqk_sb, in_=bqk_d.rearrange("(ch p) -> p ch",
                                                          p=128))
        wv_sb = consts.tile([128, KT, HPC * DK], BF16)
        nc.sync.dma_start(out=wv_sb,
                          in_=wv_d.rearrange("(kt p) c -> p kt c", p=128))
        wout_sb = consts.tile([64, HPC, D], BF16)
        nc.sync.dma_start(out=wout_sb,
                          in_=wout_d.rearrange("(h p) m -> p h m", p=64))

        # ones column for the v-augmentation (denominator row)
        ones3 = consts.tile([128, HPC], BF16)
        nc.vector.memset(ones3, 1.0)
        # lower-triangular-inclusive multiplicative mask for the diagonal
        # 128x128 block, two head-planes: tri2[p, hh, c] = 1 if p <= c else 0
        tri2 = consts.tile([128, 2, 128], BF16)
        nc.vector.memset(tri2, 1.0)
        for hh in range(2):
            nc.gpsimd.affine_select(
                out=tri2[:, hh, :], in_=tri2[:, hh, :],
                compare_op=mybir.AluOpType.is_ge,
                fill=0.0, base=0, pattern=[[1, 128]], channel_multiplier=-1)
        tri = tri2[:, 0, :]

        with tc.tile_pool(name="xtp", bufs=3) as xtp, \
             tc.tile_pool(name="work_ps", bufs=2, space="PSUM") as workp, \
             tc.tile_pool(name="stps", bufs=2, space="PSUM") as stps, \
             tc.tile_pool(name="cps", bufs=2, space="PSUM") as cpsp, \
             tc.tile_pool(name="pt", bufs=3) as ptp, \
             tc.tile_pool(name="ctxn", bufs=6) as ctxp, \
             tc.tile_pool(name="small", bufs=4) as smp, \
             tc.tile_pool(name="outsb", bufs=3) as outp:

            EXP = mybir.ActivationFunctionType.Exp
            LN = mybir.ActivationFunctionType.Ln
            xt_tiles = {}

            def emit_xt_dma(ich):
                if ich >= n_ich:
                    return
                i0 = ich * ICH_W
                xt = xtp.tile([128, KT, ICH_W], BF16, tag="xt")
                nc.sync.dma_start(
                    out=xt,
                    in_=xt_d[:, i0:i0 + ICH_W].rearrange(
                        "(kt p) i -> p kt i", p=128))
                xt_tiles[ich] = xt

            def emit_proj(ich):
                """qk^T + v projections for i-chunk ich's tokens."""
                i0 = ich * ICH_W
                xt = xt_tiles.pop(ich)
                for ch in range(4):
                    qps = workp.tile([128, 512], F32, tag="w", space="PSUM")
                    for kt in range(KT):
                        nc.tensor.matmul(
                            qps,
                            lhsT=wqk_sb[:, kt, ch * 128:(ch + 1) * 128],
                            rhs=xt[:, kt, :],
                            start=(kt == 0), stop=(kt == KT - 1),
                        )
                    nc.vector.tensor_scalar_add(
                        qk_sb[:, ch, i0:i0 + ICH_W], qps,
                        bqk_sb[:, ch:ch + 1])
                for tl in range(ICH_W // 128):
                    tch = ich * (ICH_W // 128) + tl
                    vps = workp.tile([128, 512], F32, tag="w", space="PSUM")
                    for kt in range(KT):
                        nc.tensor.matmul(
                            vps[:, 0:HPC * DK],
                            lhsT=xt[:, kt, tl * 128:(tl + 1) * 128],
                            rhs=wv_sb[:, kt, :],
                            start=(kt == 0), stop=(kt == KT - 1),
                        )
                    nc.vector.tensor_copy(
                        vaug_sb[:, tch, :, 0:DK],
                        vps[:, 0:HPC * DK].rearrange("p (h d) -> p h d",
                                                     h=HPC),
                    )
                    nc.vector.tensor_copy(
                        vaug_sb[:, tch, :, DK:DK + 1],
                        ones3.rearrange("p (a b) -> p a b", b=1))

            def normalize(cps, use_act):
                """ctx[0:64]/den[64] -> bf16 [64, ICH_W] tile."""
                recip = smp.tile([1, ICH_W], F32, tag="recip")
                if use_act:
                    lnd = smp.tile([1, ICH_W], F32, tag="lnd")
                    nc.scalar.activation(lnd, cps[64:65, :], LN,
                                         bias=0.0, scale=1.0)
                    nc.scalar.activation(recip, lnd, EXP,
                                         bias=0.0, scale=-1.0)
                else:
                    nc.vector.reciprocal(recip, cps[64:65, :])
                rb = smp.tile([64, ICH_W], F32, tag="rb")
                nc.gpsimd.partition_broadcast(rb, recip)
                cn = ctxp.tile([64, ICH_W], BF16, tag="ctxn")
                nc.vector.tensor_mul(cn, cps[0:64, :], rb)
                return cn

            def emit_outproj(ich, ctxn):
                i0 = ich * ICH_W
                for tsub in range(ICH_W // 128):
                    osb = outp.tile([128, D], BF16, tag="osb")
                    for m0, m1 in ((0, 384), (384, D)):
                        ops = workp.tile([128, 512], F32, tag="w",
                                         space="PSUM")
                        for h in range(HPC):
                            nc.tensor.matmul(
                                ops[:, 0:m1 - m0],
                                lhsT=ctxn[h][:, tsub * 128:(tsub + 1) * 128],
                                rhs=wout_sb[:, h, m0:m1],
                                start=(h == 0), stop=(h == HPC - 1))
                        nc.vector.tensor_copy(osb[:, m0:m1],
                                              ops[:, 0:m1 - m0])
                    nc.sync.dma_start(
                        out=out_d[i0 + tsub * 128:i0 + (tsub + 1) * 128, :],
                        in_=osb)

            # head views: (qT, kT) partition slices
            # h0: q=ch0[0:64]   k=ch1[0:64]
            # h1: q=ch0[64:128] k=ch1[64:128]
            # h2 even jb: q=ch2[0:64]  k=ch3[0:64]
            # h2 odd  jb: q=ch3[64:128] k=ch2[64:128]

            emit_xt_dma(0)
            emit_xt_dma(1)
            emit_proj(0)
            for ich in range(n_ich):
                i0 = ich * ICH_W
                emit_xt_dma(ich + 2)
                njb = (i0 + ICH_W) // JB_W     # causal: j-blocks 0..njb-1
                ctxn = {}

                def sw(jb):
                    s = jb - (njb - 4)          # diag position if >= 0
                    return s, (128 * s if s > 0 else 0)

                # ---- pass A: heads 0/1 row-group paired, software-
                # pipelined: scores+exp one block ahead of mask+pv ----
                cps0 = cpsp.tile([65, ICH_W], F32, tag="cps", space="PSUM")
                cps1 = cpsp.tile([65, ICH_W], F32, tag="cps", space="PSUM")

                def scores_a(jb):
                    j0 = jb * JB_W
                    s, w0 = sw(jb)
                    st = stps.tile([128, 2, ICH_W], F32, tag="st",
                                   space="PSUM")
                    nc.tensor.matmul(
                        st[:, 0, w0:],
                        lhsT=qk_sb[0:64, 1, j0:j0 + JB_W],
                        rhs=qk_sb[0:64, 0, i0 + w0:i0 + ICH_W],
                        start=True, stop=True)
                    nc.tensor.matmul(
                        st[:, 1, w0:],
                        lhsT=qk_sb[64:128, 1, j0:j0 + JB_W],
                        rhs=qk_sb[64:128, 0, i0 + w0:i0 + ICH_W],
                        start=True, stop=True)
                    pt = ptp.tile([128, 2, ICH_W], BF16, tag="pt")
                    nc.scalar.activation(pt[:, :, w0:], st[:, :, w0:], EXP,
                                         bias=0.0, scale=1.0 / np.sqrt(DK))
                    return pt

                def pv_a(jb, pt):
                    s, w0 = sw(jb)
                    if s >= 0:
                        nc.vector.tensor_mul(
                            pt[:, :, w0:w0 + 128],
                            pt[:, :, w0:w0 + 128], tri2)
                    nc.tensor.matmul(
                        cps0[:, w0:], lhsT=vaug_sb[:, jb, 0, :],
                        rhs=pt[:, 0, w0:],
                        start=(jb == 0), stop=(jb == njb - 1))
                    nc.tensor.matmul(
                        cps1[:, w0:], lhsT=vaug_sb[:, jb, 1, :],
                        rhs=pt[:, 1, w0:],
                        start=(jb == 0), stop=(jb == njb - 1))

                pend = None
                for jb in range(njb):
                    pt = scores_a(jb)
                    if pend is not None:
                        pv_a(pend[0], pend[1])
                    pend = (jb, pt)
                pv_a(pend[0], pend[1])

                # ---- normalize h0 (ACT recip: releases the PSUM slot that
                # pass B's accumulator reuses), h1 (DVE, overlaps pass B) ----
                ctxn[0] = normalize(cps0, use_act=True)
                ctxn[1] = normalize(cps1, use_act=False)

                # ---- pass B: head 2, alternating row groups, pipelined ----
                cps2 = cpsp.tile([65, ICH_W], F32, tag="cps", space="PSUM")

                def scores_b(grp):
                    st = stps.tile([128, 2, ICH_W], F32, tag="st",
                                   space="PSUM")
                    pt = ptp.tile([128, 2, ICH_W], BF16, tag="pt")
                    w0s = []
                    for jj in range(2):
                        jb = grp * 2 + jj
                        j0 = jb * JB_W
                        s, w0 = sw(jb)
                        w0s.append(w0)
                        if jb % 2 == 0:
                            lhsT = qk_sb[0:64, 3, j0:j0 + JB_W]
                            rhs = qk_sb[0:64, 2, i0 + w0:i0 + ICH_W]
                        else:
                            lhsT = qk_sb[64:128, 2, j0:j0 + JB_W]
                            rhs = qk_sb[64:128, 3, i0 + w0:i0 + ICH_W]
                        nc.tensor.matmul(st[:, jj, w0:], lhsT=lhsT, rhs=rhs,
                                         start=True, stop=True)
                    if w0s[0] == w0s[1]:
                        nc.scalar.activation(
                            pt[:, :, w0s[0]:], st[:, :, w0s[0]:], EXP,
                            bias=0.0, scale=1.0 / np.sqrt(DK))
                    else:
                        for jj in range(2):
                            nc.scalar.activation(
                                pt[:, jj, w0s[jj]:], st[:, jj, w0s[jj]:],
                                EXP, bias=0.0, scale=1.0 / np.sqrt(DK))
                    return pt

                def pv_b(grp, pt):
                    for jj in range(2):
                        jb = grp * 2 + jj
                        s, w0 = sw(jb)
                        if s >= 0:
                            nc.vector.tensor_mul(
                                pt[:, jj, w0:w0 + 128],
                                pt[:, jj, w0:w0 + 128], tri)
                        nc.tensor.matmul(
                            cps2[:, w0:], lhsT=vaug_sb[:, jb, 2, :],
                            rhs=pt[:, jj, w0:],
                            start=(jb == 0), stop=(jb == njb - 1))

                pend = None
                for grp in range(njb // 2):
                    pt = scores_b(grp)
                    if pend is not None:
                        pv_b(pend[0], pend[1])
                    pend = (grp, pt)
                pv_b(pend[0], pend[1])

                # ---- projections for the next chunk keep the PE queue fed
                # while the h2 normalize chain drains ----
                if ich + 1 < n_ich:
                    emit_proj(ich + 1)
                ctxn[2] = normalize(cps2, use_act=True)
                emit_outproj(ich, ctxn)

    nc.compile()
    return nc


def _to_bf16(a):
    return np.ascontiguousarray(np.asarray(a).astype(NPBF16))


def make_core_inputs(xt_b16, W_qkv, b_qkv, W_out, hg):
    """Host-side weight slicing/permutation for one head-group hg (0..3).

    ``xt_b16``: pre-transposed+cast [D, t] bf16 (shared across the 4 cores
    of a batch — pass the same array; no per-core copy).
    """
    heads = [hg * HPC + i for i in range(HPC)]
    # W_qkv last-dim layout: c = h*192 + s*64 + d  (s: 0=q 1=k 2=v)
    def cols(h, s):
        return slice(h * 192 + s * 64, h * 192 + s * 64 + 64)

    q = [np.asarray(W_qkv[:, cols(h, 0)]) for h in heads]
    k = [np.asarray(W_qkv[:, cols(h, 1)]) for h in heads]
    v = [np.asarray(W_qkv[:, cols(h, 2)]) for h in heads]
    bq = [np.asarray(b_qkv[cols(h, 0)], np.float32) for h in heads]

    wqk = np.concatenate([q[0], q[1], k[0], k[1], q[2], k[2], k[2], q[2]],
                         axis=1)
    z = np.zeros(64, np.float32)
    bqk = np.concatenate([bq[0], bq[1], z, z, bq[2], z, z, bq[2]]).astype(
        np.float32)
    wv = np.concatenate(v, axis=1)
    wout = np.concatenate(
        [np.asarray(W_out[h * DK:(h + 1) * DK, :]) for h in heads], axis=0)
    return {
        "xt": xt_b16,
        "wqk": _to_bf16(wqk),
        "bqk": np.ascontiguousarray(bqk),
        "wv": _to_bf16(wv),
        "wout": _to_bf16(wout),
    }


_CACHE = {}


def _get_program(t=T):
    if t not in _CACHE:
        _CACHE[t] = build_program(t)
    return _CACHE[t]


def run_cores(inputs, t=T, trace=False):
    nc = _get_program(t)
    x = np.asarray(inputs["x"], np.float32)
    xt_b16 = [np.ascontiguousarray(x[b].T.astype(NPBF16)) for b in range(B)]
    in_maps = []
    for core in range(N_CORES):
        b, hg = core // 4, core % 4
        in_maps.append(make_core_inputs(xt_b16[b], inputs["W_qkv"],
                                        inputs["b_qkv"], inputs["W_out"], hg))
    res = run_bass_kernel_spmd(nc, in_maps, list(range(N_CORES)), trace=trace)
    return res


def gather(inputs, results):
    b_qkv = np.asarray(inputs["b_qkv"], np.float32)
    W_out = np.asarray(inputs["W_out"], np.float32)
    b_out = np.asarray(inputs["b_out"], np.float32)
    bv = np.concatenate([b_qkv[h * 192 + 128:h * 192 + 192] for h in range(H)])
    fold = bv @ W_out + b_out                      # [D]
    t = results[0]["out"].shape[0]
    out = np.zeros((B, t, D), np.float32)
    for core in range(N_CORES):
        out[core // 4] += np.asarray(results[core]["out"], np.float32)
    out += fold[None, None, :]
    return out


def kernel(**inputs):
    res = run_cores(inputs)
    return gather(inputs, res.results)


if __name__ == "__main__":
    # smoke test with random data
    rng = np.random.default_rng(0)
    inputs = {
        "x": rng.standard_normal((B, T, D), dtype=np.float32),
        "mask": np.triu(np.ones((T, T), dtype=bool), k=1),
        "W_qkv": (rng.standard_normal((D, 3 * D), dtype=np.float32)
                  / np.sqrt(D)),
        "b_qkv": rng.standard_normal(3 * D).astype(np.float32) * 0.02,
        "W_out": (rng.standard_normal((D, D), dtype=np.float32)
                  / np.sqrt(D)),
        "b_out": rng.standard_normal(D).astype(np.float32) * 0.02,
    }
    out = kernel(**inputs)
    print(out.shape, out.dtype)
